# revision 3
# baseline (speedup 1.0000x reference)
"""Squared Euclidean distance matrix kernel for Trainium2 (8 NeuronCores).

out[i, j] = ||mat_1[i] - mat_2[j]||^2 = sq1[i] + sq2[j] - 2 * mat_1[i].mat_2[j]

Sharding: rows of mat_1 (= rows of the output) split across 8 cores;
mat_2 replicated. Each core computes a [1024, 8192] tile of the output.

Per-core dataflow:
  - The -2*cross GEMM runs in fp8e4m3 with perf_mode=DoubleRow (2 K-rows per
    cell per cycle -> ~2x the bf16 stream rate). Operands are shipped from
    host as stacked K-halves [64, 2, n] so the DR access pattern is a plain
    3D slice. fp8 feeds ONLY the big GEMM; the sq1/sq2 row-norm terms come
    from separate bf16 copies of the inputs (fp8-derived norms would blow the
    2e-2 error budget; measured rel err of this mix is ~1.2e-2).
  - sq1[m] = 0.25*colsum(m1ts^2), sq2[n] = colsum(m2t^2): DVE/ACT squares +
    ones-matmuls on PE. Each per-chunk [1,512] colsum lands in a distinct
    psum partition (shifted one-hot stationary Woh) so a whole bank drains
    with ONE [8,512] copy instead of eight 1-partition copies.
  - Per output tile pair [128 x 1024] (one 2-bank psum tile):
      psum  = m1d8.T @ m2d8 (fp8 DR, K=128 as 64x2)   x2 columns-halves
      psum += [ones; sq1].T @ [sq2; ones]  (K=2 fp16)  x2
      one [128,1024] copy psum -> fp16 SBUF staging (ScalarE/VectorE alt.)
      DMA staging -> DRAM in [128, 4096] pieces
  - Column-half loop (g) runs OUTER so the first half's mm2 only needs the
    first sq2 batch -- the second input half streams in under the main loop.
  Keeping the PE gap-free matters: the HAM clock gate runs the PE at 1.2 GHz
  until it sees ~3.4us of sustained busy, 2.4 GHz after.
"""

import sys

import numpy as np

if "/opt/trn_rl_repo" not in sys.path:
    sys.path.insert(0, "/opt/trn_rl_repo")

import concourse.bass as bass
import concourse.mybir as mybir
import concourse.tile as tile
from concourse.bass_utils import run_bass_kernel_spmd

N1, N2, D = 8192, 8192, 128
NCORES = 8
MS = N1 // NCORES  # 1024 output rows per core

F32 = mybir.dt.float32
BF16 = mybir.dt.bfloat16
F16 = mybir.dt.float16
F8 = mybir.dt.float8e4


def legalize_waits(nc):
    """Split multi-wait instructions into single-wait NoOps.

    The TPB ISA encodes exactly one sync-wait per instruction
    (NEURON_ISA_TPB_EVENTS has a single wait slot) and this walrus build
    refuses instructions carrying more ("Too many sync wait commands").
    Tile emits multi-wait sync_info freely (e.g. the kernel-tail drain waits
    on every active proc). Semantics are preserved by having the same engine
    execute one NoOp per extra wait immediately before the instruction.
    """
    n = 0
    for fn in nc.m.functions:
        for blk in fn.blocks:
            new_list = []
            changed = False
            for inst in blk.instructions:
                si = inst.sync_info
                waits = list(si.on_wait) if si and si.on_wait else []
                if len(waits) > 1:
                    changed = True
                    for w in waits[:-1]:
                        nop = mybir.InstNoOp(name=f"I-wsplit-{n}", ins=[], outs=[])
                        n += 1
                        nop.engine = inst.engine
                        nop.sync_info = mybir.SyncInfo(on_wait=[w], on_update=[])
                        new_list.append(nop)
                    si.on_wait = [waits[-1]]
                    inst.sync_info = si
                new_list.append(inst)
            if changed:
                blk.instructions = new_list
    return nc


def build_nc(ms=MS, n2=N2, d=D, legalize=True, use_fp8=True):
    """Build the per-core Bass module. All cores run the same program (SPMD);
    the mat_1 shard differs per core via in_maps."""
    assert ms % 512 == 0 and n2 % 4096 == 0 and d == 128
    n_mb = ms // 128    # M blocks of 128 rows
    n_nb = n2 // 512    # N blocks of 512 cols
    n_g = n_nb // 8     # column-half supergroups (8 nj each)
    DR = mybir.MatmulPerfMode.DoubleRow

    nc = bass.Bass()
    m1d8 = nc.declare_dram_parameter("m1d8", [64, 2, ms], F8, isOutput=False)
    m2d8 = nc.declare_dram_parameter("m2d8", [64, 2, n2], F8, isOutput=False)
    m1ts = nc.declare_dram_parameter("m1ts", [d, ms], BF16, isOutput=False)
    m2t = nc.declare_dram_parameter("m2t", [d, n2], BF16, isOutput=False)
    out = nc.declare_dram_parameter("out", [ms, n2], F16, isOutput=True)

    with tile.TileContext(nc) as tc:
        with (
            tc.tile_pool(name="big", bufs=1) as big,
            tc.tile_pool(name="scratch", bufs=3) as scr,
            tc.tile_pool(name="stage", bufs=3) as stagep,
            tc.tile_pool(name="psum", bufs=4, space="PSUM") as psump,
        ):
            # ---- input loads; mm1-critical fp8 via the ACT HWDGE queue,
            #      sq-critical bf16 via the Sync queue (parallel issue) ----
            M1D8 = big.tile([64, 2, ms], F8, tag="m1d8")
            nc.scalar.dma_start(out=M1D8[:], in_=m1d8[:])
            M2D8 = big.tile([64, 2, n2], F8, tag="m2d8")
            for h0 in range(0, n2, 4096):
                nc.scalar.dma_start(
                    out=M2D8[:, :, h0 : h0 + 4096], in_=m2d8[:, :, h0 : h0 + 4096]
                )
            M1TS = big.tile([d, ms], BF16, tag="m1ts")
            nc.sync.dma_start(out=M1TS[:], in_=m1ts[:])
            M2T = big.tile([d, n2], BF16, tag="m2t")
            for c0 in range(0, n2, 2048):
                nc.sync.dma_start(out=M2T[:, c0 : c0 + 2048], in_=m2t[:, c0 : c0 + 2048])

            # ---- constants (tiny memsets + DMA broadcast; avoids 1-partition
            #      memsets which cost (120+FD)/0.96 ns on DVE) ----
            onesA = big.tile([128, 64], F16, tag="onesA")
            nc.vector.memset(onesA[:], 1.0)
            # Shifted one-hot stationary: Woh[:, 8] = 1, rest 0. sq-matmul c
            # uses lhsT = Woh[:, 8-c : 16-c] so its colsum lands in partition c.
            Woh = big.tile([128, 17], F16, tag="woh")
            nc.vector.memset(Woh[:], 0.0)
            nc.vector.memset(Woh[:, 8:9], 1.0)

            # rank-2 matmul operands: LHS2 = [ones; sq1], RHS2 = [sq2; ones]
            LHS2 = big.tile([2, ms], F16, tag="lhs2")
            nc.sync.dma_start(out=LHS2[0:1, :], in_=onesA[:, 0 : ms // 128])
            RHS2 = big.tile([2, n2], F16, tag="rhs2")
            nc.sync.dma_start(out=RHS2[1:2, :], in_=onesA[:, 0 : n2 // 128])

            # ---- sq1 = 0.25 * colsum(m1ts^2)   (m1ts = -2*m1^T) ----
            n_c1 = ms // 512
            sq1_scr = scr.tile([d, ms], F16, tag="sq_scr")
            nc.vector.tensor_mul(sq1_scr[:], M1TS[:], M1TS[:])
            ps_sq1 = psump.tile([8, 512], F32, tag="ps")
            for c in range(n_c1):
                nc.tensor.matmul(
                    ps_sq1[:],
                    Woh[:, 8 - c : 16 - c],
                    sq1_scr[:, c * 512 : (c + 1) * 512],
                    start=(c == 0),
                    stop=(c == n_c1 - 1),
                )
            sq1_st = scr.tile([8, 512], F16, tag="sq1_st")
            nc.scalar.mul(sq1_st[:n_c1, :], ps_sq1[:n_c1, :], 0.25)
            nc.sync.dma_start(out=LHS2[1:2, :], in_=sq1_st[:n_c1, :])

            # ---- sq2 = colsum(m2t^2), batches of 8 chunks (4096 cols) ----
            for b in range(n2 // 4096):
                sq_scr = scr.tile([d, 4096], F16, tag="sq2_scr")
                for k in range(2):
                    c0 = b * 4096 + k * 2048
                    if k == 0:
                        nc.vector.tensor_mul(
                            sq_scr[:, :2048], M2T[:, c0 : c0 + 2048], M2T[:, c0 : c0 + 2048]
                        )
                    else:
                        nc.scalar.square(sq_scr[:, 2048:], M2T[:, c0 : c0 + 2048])
                ps_b = psump.tile([8, 512], F32, tag="ps")
                for c in range(8):
                    nc.tensor.matmul(
                        ps_b[:],
                        Woh[:, 8 - c : 16 - c],
                        sq_scr[:, c * 512 : (c + 1) * 512],
                        start=(c == 0),
                        stop=(c == 7),
                    )
                st_b = scr.tile([8, 512], F16, tag="sq2_st")
                if b % 2 == 0:
                    nc.vector.tensor_copy(st_b[:], ps_b[:])
                else:
                    nc.scalar.copy(st_b[:], ps_b[:])
                nc.sync.dma_start(
                    out=RHS2[0:1, b * 4096 : (b + 1) * 4096], in_=st_b[:]
                )

            # ---- main loop; column-half g OUTER so g=0 only needs the first
            #      input/sq2 halves and starts while the rest streams in ----
            for g in range(n_g):
                for mi in range(n_mb):
                    r0 = mi * 128
                    pss = []
                    for t in range(4):
                        ps = psump.tile([128, 1024], F32, tag="ps")
                        for h in range(2):
                            nj = g * 8 + t * 2 + h
                            c0 = nj * 512
                            if use_fp8:
                                nc.tensor.matmul(
                                    ps[:, h * 512 : (h + 1) * 512],
                                    M1D8[:, :, r0 : r0 + 128],
                                    M2D8[:, :, c0 : c0 + 512],
                                    start=True,
                                    stop=False,
                                    perf_mode=DR,
                                )
                            else:
                                nc.tensor.matmul(
                                    ps[:, h * 512 : (h + 1) * 512],
                                    M1TS[:, r0 : r0 + 128],
                                    M2T[:, c0 : c0 + 512],
                                    start=True,
                                    stop=False,
                                )
                        pss.append(ps)
                    for t, ps in enumerate(pss):
                        for h in range(2):
                            nj = g * 8 + t * 2 + h
                            c0 = nj * 512
                            nc.tensor.matmul(
                                ps[:, h * 512 : (h + 1) * 512],
                                LHS2[:, r0 : r0 + 128],
                                RHS2[:, c0 : c0 + 512],
                                start=False,
                                stop=True,
                            )
                    stage = stagep.tile([128, 4096], F16, tag="stage")
                    for t, ps in enumerate(pss):
                        dst = stage[:, t * 1024 : (t + 1) * 1024]
                        if t % 2 == 0:
                            nc.scalar.copy(dst, ps[:])
                        else:
                            nc.vector.tensor_copy(dst, ps[:])
                    nc.sync.dma_start(
                        out=out[r0 : r0 + 128, g * 4096 : (g + 1) * 4096],
                        in_=stage[:],
                    )
    return legalize_waits(nc) if legalize else nc


_NC_CACHE = {}


def _get_nc(ms=MS, n2=N2, d=D):
    key = (ms, n2, d)
    if key not in _NC_CACHE:
        _NC_CACHE[key] = build_nc(ms, n2, d)
    return _NC_CACHE[key]


def _prep_inputs(m1, m2, ms):
    """Host-side layout/precision prep (transpose, dtype casts, K-half stack)."""
    bf16 = mybir.dt.np(BF16)
    f8 = mybir.dt.np(F8)
    a = np.ascontiguousarray(-2.0 * m1.T)          # [128, n1] f32
    b = np.ascontiguousarray(m2.T)                 # [128, n2] f32
    m1ts = a.astype(bf16)
    m2t = b.astype(bf16)
    a8 = a.astype(f8)
    b8 = b.astype(f8)
    m2d8 = np.ascontiguousarray(np.stack([b8[:64], b8[64:]], axis=1))  # [64,2,n2]
    ncores = a.shape[1] // ms
    per_core = []
    for c in range(ncores):
        a8c = a8[:, c * ms : (c + 1) * ms]
        per_core.append(
            {
                "m1d8": np.ascontiguousarray(np.stack([a8c[:64], a8c[64:]], axis=1)),
                "m1ts": np.ascontiguousarray(m1ts[:, c * ms : (c + 1) * ms]),
                "m2d8": m2d8,
                "m2t": m2t,
            }
        )
    return per_core


def kernel(mat_1, mat_2, _trace=False):
    m1 = np.ascontiguousarray(np.asarray(mat_1, dtype=np.float32))
    m2 = np.ascontiguousarray(np.asarray(mat_2, dtype=np.float32))
    assert m1.shape == (N1, D) and m2.shape == (N2, D)

    in_maps = _prep_inputs(m1, m2, MS)
    nc = _get_nc()
    r = run_bass_kernel_spmd(nc, in_maps, list(range(NCORES)), trace=_trace)
    out = np.concatenate(
        [r.results[c]["out"].astype(np.float32) for c in range(NCORES)], axis=0
    )
    if _trace:
        return out, r.exec_time_ns
    return out


# revision 4
# speedup vs baseline: 1.3953x; 1.3953x over previous
"""Squared Euclidean distance matrix kernel for Trainium2 (8 NeuronCores).

out[i, j] = ||mat_1[i] - mat_2[j]||^2 = sq1[i] + sq2[j] - 2 * mat_1[i].mat_2[j]

Sharding: rows of mat_1 (= rows of the output) split across 8 cores;
mat_2 replicated. Each core computes a [1024, 8192] tile of the output.

Per-core dataflow (cross GEMM in bf16; output written fp16, upcast on host —
fp16 quantization adds ~2e-3 vs the 2e-2 gate; fp8 was tried and measured:
DoubleRow gives NO stream speedup at K=128 — the PE stream is column-rate
limited, so fp8 only pays at K>=256):
  - Host pre-transposes inputs so the contraction dim (d=128) lands on SBUF
    partitions and folds the -2 scale into m1ts (layout/scale prep only).
  - sq1[m] = 0.25*colsum(m1ts^2), sq2[n] = colsum(m2t^2): squares on DVE
    (tensor_mul -- keeps ScalarE on Copy-only so no ACT table load), colsums
    via ones-matmuls on PE. Each per-chunk [1,512] colsum lands in a distinct
    psum partition (shifted one-hot stationary Woh) so a whole batch drains
    with ONE [8,512] copy instead of eight 1-partition copies.
  - Main loop, group-of-6 psum banks (2 banks reserved for the sq phase so
    the first mm1 group interleaves with sq matmuls):
      psum  = m1ts.T @ m2t_block           (K=128 bf16 matmul, -2*cross)
      psum += [ones; sq1].T @ [sq2; ones]  (K=2 fp16 matmul, adds sq1+sq2)
      copy psum -> fp16 SBUF staging (ScalarE / VectorE alternating)
      DMA staging -> DRAM in [128, 3072] pieces
  - Column-group loop runs OUTER (grp, then mi) so the first groups only
    need the first sq2 batch; the rest of the input streams in underneath.
  - Input DMAs split across the two HWDGE queues (Sync + ScalarE) so issue
    (~0.7us per DMA instruction) doesn't serialize the critical path.
  Keeping the PE gap-free matters: the HAM clock gate runs the PE at 1.2 GHz
  until it sees ~3.4us of sustained busy, 2.4 GHz after.
"""

import sys

import numpy as np

if "/opt/trn_rl_repo" not in sys.path:
    sys.path.insert(0, "/opt/trn_rl_repo")

import concourse.bass as bass
import concourse.mybir as mybir
import concourse.tile as tile
from concourse.bass_utils import run_bass_kernel_spmd

N1, N2, D = 8192, 8192, 128
NCORES = 8
MS = N1 // NCORES  # 1024 output rows per core

F32 = mybir.dt.float32
BF16 = mybir.dt.bfloat16
F16 = mybir.dt.float16


def legalize_waits(nc):
    """Split multi-wait instructions into single-wait NoOps.

    The TPB ISA encodes exactly one sync-wait per instruction
    (NEURON_ISA_TPB_EVENTS has a single wait slot) and this walrus build
    refuses instructions carrying more ("Too many sync wait commands").
    Tile emits multi-wait sync_info freely (e.g. the kernel-tail drain waits
    on every active proc). Semantics are preserved by having the same engine
    execute one NoOp per extra wait immediately before the instruction.
    """
    n = 0
    for fn in nc.m.functions:
        for blk in fn.blocks:
            new_list = []
            changed = False
            for inst in blk.instructions:
                si = inst.sync_info
                waits = list(si.on_wait) if si and si.on_wait else []
                if len(waits) > 1:
                    changed = True
                    for w in waits[:-1]:
                        nop = mybir.InstNoOp(name=f"I-wsplit-{n}", ins=[], outs=[])
                        n += 1
                        nop.engine = inst.engine
                        nop.sync_info = mybir.SyncInfo(on_wait=[w], on_update=[])
                        new_list.append(nop)
                    si.on_wait = [waits[-1]]
                    inst.sync_info = si
                new_list.append(inst)
            if changed:
                blk.instructions = new_list
    return nc


def build_nc(ms=MS, n2=N2, d=D, legalize=True):
    """Build the per-core Bass module. All cores run the same program (SPMD);
    the mat_1 shard differs per core via in_maps."""
    assert ms % 512 == 0 and n2 % 4096 == 0 and d == 128
    n_mb = ms // 128    # M blocks of 128 rows
    n_nb = n2 // 512    # N blocks of 512 cols
    GROUP = 6           # psum banks per main matmul group (2 reserved for sq)
    grps = [list(range(g0, min(g0 + GROUP, n_nb))) for g0 in range(0, n_nb, GROUP)]

    nc = bass.Bass()
    m1ts = nc.declare_dram_parameter("m1ts", [d, ms], BF16, isOutput=False)
    m2t = nc.declare_dram_parameter("m2t", [d, n2], BF16, isOutput=False)
    out = nc.declare_dram_parameter("out", [ms, n2], F16, isOutput=True)

    with tile.TileContext(nc) as tc:
        with (
            tc.tile_pool(name="big", bufs=1) as big,
            tc.tile_pool(name="scratch", bufs=2) as scr,
            tc.tile_pool(name="sqst", bufs=3) as sqstp,
            tc.tile_pool(name="stage", bufs=3) as stagep,
            tc.tile_pool(name="psA", bufs=GROUP, space="PSUM") as psA,
            tc.tile_pool(name="psB", bufs=2, space="PSUM") as psB,
        ):
            # ---- input loads, split across both HWDGE queues ----
            M1TS = big.tile([d, ms], BF16, tag="m1ts")
            M2T = big.tile([d, n2], BF16, tag="m2t")
            nc.sync.dma_start(out=M1TS[:], in_=m1ts[:])
            nc.sync.dma_start(out=M2T[:, 0:2048], in_=m2t[:, 0:2048])
            nc.scalar.dma_start(out=M2T[:, 2048:4096], in_=m2t[:, 2048:4096])
            nc.sync.dma_start(out=M2T[:, 4096:6144], in_=m2t[:, 4096:6144])
            nc.scalar.dma_start(out=M2T[:, 6144:8192], in_=m2t[:, 6144:8192])

            # ---- constants (tiny memsets + DMA broadcast; avoids 1-partition
            #      memsets which cost (120+FD)/0.96 ns on DVE) ----
            onesA = big.tile([128, 64], F16, tag="onesA")
            nc.vector.memset(onesA[:], 1.0)
            # Shifted one-hot stationary: Woh[:, 8] = 1, rest 0. sq-matmul c
            # uses lhsT = Woh[:, 8-c : 16-c] so its colsum lands in partition c.
            Woh = big.tile([128, 17], F16, tag="woh")
            nc.vector.memset(Woh[:], 0.0)
            nc.vector.memset(Woh[:, 8:9], 1.0)

            # rank-2 matmul operands: LHS2 = [ones; sq1], RHS2 = [sq2; ones]
            LHS2 = big.tile([2, ms], F16, tag="lhs2")
            nc.sync.dma_start(out=LHS2[0:1, :], in_=onesA[:, 0 : ms // 128])
            RHS2 = big.tile([2, n2], F16, tag="rhs2")
            nc.sync.dma_start(out=RHS2[1:2, :], in_=onesA[:, 0 : n2 // 128])

            # ---- sq1 = 0.25 * colsum(m1ts^2)   (m1ts = -2*m1^T) ----
            n_c1 = ms // 512
            sq1_scr = scr.tile([d, ms], F16, tag="sq1_scr")
            nc.vector.tensor_mul(sq1_scr[:], M1TS[:], M1TS[:])
            ps_sq1 = psB.tile([8, 512], F32, tag="ps")
            for c in range(n_c1):
                nc.tensor.matmul(
                    ps_sq1[:],
                    Woh[:, 8 - c : 16 - c],
                    sq1_scr[:, c * 512 : (c + 1) * 512],
                    start=(c == 0),
                    stop=(c == n_c1 - 1),
                )
            sq1_st = sqstp.tile([8, 512], F16, tag="sq1_st")
            nc.scalar.mul(sq1_st[:n_c1, :], ps_sq1[:n_c1, :], 0.25)
            nc.sync.dma_start(out=LHS2[1:2, :], in_=sq1_st[:n_c1, :])

            def sq2_batch(b):
                """sq2 for columns [b*4096, (b+1)*4096) -> RHS2 row 0."""
                sq_scr = scr.tile([d, 4096], F16, tag="sq2_scr")
                for k in range(2):
                    c0 = b * 4096 + k * 2048
                    nc.vector.tensor_mul(
                        sq_scr[:, k * 2048 : (k + 1) * 2048],
                        M2T[:, c0 : c0 + 2048],
                        M2T[:, c0 : c0 + 2048],
                    )
                ps_b = psB.tile([8, 512], F32, tag="ps")
                for c in range(8):
                    nc.tensor.matmul(
                        ps_b[:],
                        Woh[:, 8 - c : 16 - c],
                        sq_scr[:, c * 512 : (c + 1) * 512],
                        start=(c == 0),
                        stop=(c == 7),
                    )
                st_b = sqstp.tile([8, 512], F16, tag="sq2_st")
                nc.scalar.copy(st_b[:], ps_b[:])
                nc.sync.dma_start(
                    out=RHS2[0:1, b * 4096 : (b + 1) * 4096], in_=st_b[:]
                )

            def mm1s(gjs, r0):
                pss = []
                for nj in gjs:
                    c0 = nj * 512
                    ps = psA.tile([128, 512], F32, tag="ps")
                    nc.tensor.matmul(
                        ps[:],
                        M1TS[:, r0 : r0 + 128],
                        M2T[:, c0 : c0 + 512],
                        start=True,
                        stop=False,
                    )
                    pss.append(ps)
                return pss

            def mm2s_and_out(gjs, pss, r0):
                for nj, ps in zip(gjs, pss):
                    c0 = nj * 512
                    nc.tensor.matmul(
                        ps[:],
                        LHS2[:, r0 : r0 + 128],
                        RHS2[:, c0 : c0 + 512],
                        start=False,
                        stop=True,
                    )
                g0 = gjs[0] * 512
                gw = len(gjs) * 512
                stage = stagep.tile([128, 3072], F16, tag="stage")
                for idx, (nj, ps) in enumerate(zip(gjs, pss)):
                    dst = stage[:, idx * 512 : (idx + 1) * 512]
                    if idx % 2 == 0:
                        nc.scalar.copy(dst, ps[:])
                    else:
                        nc.vector.tensor_copy(dst, ps[:])
                nc.sync.dma_start(
                    out=out[r0 : r0 + 128, g0 : g0 + gw], in_=stage[:, :gw]
                )

            # ---- preamble interleave: first main group's mm1s run between
            #      the sq matmuls so the PE warms early and never idles ----
            pss0 = mm1s(grps[0], 0)
            sq2_batch(0)
            mm2s_and_out(grps[0], pss0, 0)
            sq2_batch(1)

            # ---- main loop, grp outer / mi inner ----
            for gi, gjs in enumerate(grps):
                for mi in range(n_mb):
                    if gi == 0 and mi == 0:
                        continue  # emitted above
                    r0 = mi * 128
                    pss = mm1s(gjs, r0)
                    mm2s_and_out(gjs, pss, r0)
    return legalize_waits(nc) if legalize else nc


_NC_CACHE = {}


def _get_nc(ms=MS, n2=N2, d=D):
    key = (ms, n2, d)
    if key not in _NC_CACHE:
        _NC_CACHE[key] = build_nc(ms, n2, d)
    return _NC_CACHE[key]


def _prep_inputs(m1, m2, ms):
    """Host-side layout/precision prep (transpose + dtype casts only)."""
    bf16 = mybir.dt.np(BF16)
    m1ts = np.ascontiguousarray(-2.0 * m1.T).astype(bf16)  # [128, n1]
    m2t = np.ascontiguousarray(m2.T).astype(bf16)          # [128, n2]
    ncores = m1ts.shape[1] // ms
    return [
        {
            "m1ts": np.ascontiguousarray(m1ts[:, c * ms : (c + 1) * ms]),
            "m2t": m2t,
        }
        for c in range(ncores)
    ]


def kernel(mat_1, mat_2, _trace=False):
    m1 = np.ascontiguousarray(np.asarray(mat_1, dtype=np.float32))
    m2 = np.ascontiguousarray(np.asarray(mat_2, dtype=np.float32))
    assert m1.shape == (N1, D) and m2.shape == (N2, D)

    in_maps = _prep_inputs(m1, m2, MS)
    nc = _get_nc()
    r = run_bass_kernel_spmd(nc, in_maps, list(range(NCORES)), trace=_trace)
    out = np.concatenate(
        [r.results[c]["out"].astype(np.float32) for c in range(NCORES)], axis=0
    )
    if _trace:
        return out, r.exec_time_ns
    return out


# revision 7
# speedup vs baseline: 1.7525x; 1.2559x over previous
"""Squared Euclidean distance matrix kernel for Trainium2 (8 NeuronCores).

out[i, j] = ||mat_1[i] - mat_2[j]||^2 = sq1[i] + sq2[j] - 2 * mat_1[i].mat_2[j]

Sharding: rows of mat_1 (= rows of the output) split across 8 cores;
mat_2 replicated. Each core computes a [1024, 8192] tile of the output.

Per-core dataflow (cross GEMM in bf16; output written fp16, upcast on host —
fp16 quantization adds ~2e-3 vs the 2e-2 gate; fp8 was tried and measured:
DoubleRow gives NO stream speedup at K=128 — the PE stream is column-rate
limited, so fp8 only pays at K>=256):
  - Host pre-transposes inputs so the contraction dim (d=128) lands on SBUF
    partitions and folds the -2 scale into m1ts (layout/scale prep only).
  - sq1[m] = 0.25*colsum(m1ts^2), sq2[n] = colsum(m2t^2): squares on DVE
    (tensor_mul -- keeps ScalarE on Copy-only so no ACT table load), colsums
    via ones-matmuls on PE. Each per-chunk [1,512] colsum lands in a distinct
    psum partition (shifted one-hot stationary Woh) so a whole batch drains
    with ONE [8,512] copy instead of eight 1-partition copies.
  - Main loop, group-of-6 psum banks (2 banks reserved for the sq phase so
    the first mm1 group interleaves with sq matmuls):
      psum  = m1ts.T @ m2t_block           (K=128 bf16 matmul, -2*cross)
      psum += [ones; sq1].T @ [sq2; ones]  (K=2 fp16 matmul, adds sq1+sq2)
      copy psum -> fp16 SBUF staging (ScalarE / VectorE alternating)
      DMA staging -> DRAM in [128, 3072] pieces
  - Column-group loop runs OUTER (grp, then mi) so the first groups only
    need the first sq2 batch; the rest of the input streams in underneath.
  - Input DMAs split across the two HWDGE queues (Sync + ScalarE) so issue
    (~0.7us per DMA instruction) doesn't serialize the critical path.
  Keeping the PE gap-free matters: the HAM clock gate runs the PE at 1.2 GHz
  until it sees ~3.4us of sustained busy, 2.4 GHz after.
"""

import sys

import numpy as np

if "/opt/trn_rl_repo" not in sys.path:
    sys.path.insert(0, "/opt/trn_rl_repo")

import concourse.bass as bass
import concourse.mybir as mybir
import concourse.tile as tile
from concourse.bass_utils import run_bass_kernel_spmd

N1, N2, D = 8192, 8192, 128
NCORES = 8
MS = N1 // NCORES  # 1024 output rows per core

F32 = mybir.dt.float32
BF16 = mybir.dt.bfloat16
F16 = mybir.dt.float16


def legalize_waits(nc):
    """Split multi-wait instructions into single-wait NoOps.

    The TPB ISA encodes exactly one sync-wait per instruction
    (NEURON_ISA_TPB_EVENTS has a single wait slot) and this walrus build
    refuses instructions carrying more ("Too many sync wait commands").
    Tile emits multi-wait sync_info freely (e.g. the kernel-tail drain waits
    on every active proc). Semantics are preserved by having the same engine
    execute one NoOp per extra wait immediately before the instruction.
    """
    n = 0
    for fn in nc.m.functions:
        for blk in fn.blocks:
            new_list = []
            changed = False
            for inst in blk.instructions:
                si = inst.sync_info
                waits = list(si.on_wait) if si and si.on_wait else []
                if len(waits) > 1:
                    changed = True
                    for w in waits[:-1]:
                        nop = mybir.InstNoOp(name=f"I-wsplit-{n}", ins=[], outs=[])
                        n += 1
                        nop.engine = inst.engine
                        nop.sync_info = mybir.SyncInfo(on_wait=[w], on_update=[])
                        new_list.append(nop)
                    si.on_wait = [waits[-1]]
                    inst.sync_info = si
                new_list.append(inst)
            if changed:
                blk.instructions = new_list
    return nc


def build_nc(ms=MS, n2=N2, d=D, legalize=True):
    """Build the per-core Bass module. All cores run the same program (SPMD);
    the mat_1 shard differs per core via in_maps."""
    assert ms % 512 == 0 and n2 % 4096 == 0 and d == 128
    n_mb = ms // 128    # M blocks of 128 rows
    n_nb = n2 // 512    # N blocks of 512 cols
    GROUP = 8           # psum banks per main matmul group (shared pool: the 3
                        # sq-phase [8,512] tiles rotate through the same 8
                        # bufs, so no banks sit idle during the main loop)
    grps = [list(range(g0, min(g0 + GROUP, n_nb))) for g0 in range(0, n_nb, GROUP)]

    nc = bass.Bass()
    m1ts = nc.declare_dram_parameter("m1ts", [d, ms], BF16, isOutput=False)
    m2t = nc.declare_dram_parameter("m2t", [d, n2], BF16, isOutput=False)
    out = nc.declare_dram_parameter("out", [ms, n2], F16, isOutput=True)

    with tile.TileContext(nc) as tc:
        with (
            tc.tile_pool(name="big", bufs=1) as big,
            tc.tile_pool(name="scratch", bufs=2) as scr,
            tc.tile_pool(name="sqst", bufs=3) as sqstp,
            tc.tile_pool(name="stage", bufs=3) as stagep,
            tc.tile_pool(name="psA", bufs=GROUP, space="PSUM") as psA,
        ):
            # ---- input loads, split across both HWDGE queues ----
            M1TS = big.tile([d, ms], BF16, tag="m1ts")
            M2T = big.tile([d, n2], BF16, tag="m2t")
            nc.sync.dma_start(out=M1TS[:], in_=m1ts[:])
            nc.sync.dma_start(out=M2T[:, 0:2048], in_=m2t[:, 0:2048])
            nc.scalar.dma_start(out=M2T[:, 2048:4096], in_=m2t[:, 2048:4096])
            nc.sync.dma_start(out=M2T[:, 4096:6144], in_=m2t[:, 4096:6144])
            nc.scalar.dma_start(out=M2T[:, 6144:8192], in_=m2t[:, 6144:8192])

            # ---- constants (tiny memsets + DMA broadcast; avoids 1-partition
            #      memsets which cost (120+FD)/0.96 ns on DVE) ----
            onesA = big.tile([128, 64], F16, tag="onesA")
            nc.vector.memset(onesA[:], 1.0)
            # Shifted one-hot stationary: Woh[:, 8] = 1, rest 0. sq-matmul c
            # uses lhsT = Woh[:, 8-c : 16-c] so its colsum lands in partition c.
            Woh = big.tile([128, 17], F16, tag="woh")
            nc.vector.memset(Woh[:], 0.0)
            nc.vector.memset(Woh[:, 8:9], 1.0)

            # rank-2 matmul operands: LHS2 = [ones; sq1], RHS2 = [sq2; ones]
            LHS2 = big.tile([2, ms], F16, tag="lhs2")
            nc.sync.dma_start(out=LHS2[0:1, :], in_=onesA[:, 0 : ms // 128])
            RHS2 = big.tile([2, n2], F16, tag="rhs2")
            nc.sync.dma_start(out=RHS2[1:2, :], in_=onesA[:, 0 : n2 // 128])

            # ---- sq1 = 0.25 * colsum(m1ts^2)   (m1ts = -2*m1^T) ----
            n_c1 = ms // 512
            sq1_scr = scr.tile([d, ms], F16, tag="sq1_scr")
            nc.vector.tensor_mul(sq1_scr[:], M1TS[:], M1TS[:])
            ps_sq1 = psA.tile([8, 512], F32, tag="ps")
            for c in range(n_c1):
                nc.tensor.matmul(
                    ps_sq1[:],
                    Woh[:, 8 - c : 16 - c],
                    sq1_scr[:, c * 512 : (c + 1) * 512],
                    start=(c == 0),
                    stop=(c == n_c1 - 1),
                )
            sq1_st = sqstp.tile([8, 512], F16, tag="sq1_st")
            nc.scalar.mul(sq1_st[:n_c1, :], ps_sq1[:n_c1, :], 0.25)
            nc.sync.dma_start(out=LHS2[1:2, :], in_=sq1_st[:n_c1, :])

            def sq2_batch(b):
                """sq2 for columns [b*4096, (b+1)*4096) -> RHS2 row 0."""
                sq_scr = scr.tile([d, 4096], F16, tag="sq2_scr")
                for k in range(2):
                    c0 = b * 4096 + k * 2048
                    nc.vector.tensor_mul(
                        sq_scr[:, k * 2048 : (k + 1) * 2048],
                        M2T[:, c0 : c0 + 2048],
                        M2T[:, c0 : c0 + 2048],
                    )
                ps_b = psA.tile([8, 512], F32, tag="ps")
                for c in range(8):
                    nc.tensor.matmul(
                        ps_b[:],
                        Woh[:, 8 - c : 16 - c],
                        sq_scr[:, c * 512 : (c + 1) * 512],
                        start=(c == 0),
                        stop=(c == 7),
                    )
                st_b = sqstp.tile([8, 512], F16, tag="sq2_st")
                nc.scalar.copy(st_b[:], ps_b[:])
                nc.sync.dma_start(
                    out=RHS2[0:1, b * 4096 : (b + 1) * 4096], in_=st_b[:]
                )

            def mm1s(gjs, r0):
                pss = []
                for nj in gjs:
                    c0 = nj * 512
                    ps = psA.tile([128, 512], F32, tag="ps")
                    nc.tensor.matmul(
                        ps[:],
                        M1TS[:, r0 : r0 + 128],
                        M2T[:, c0 : c0 + 512],
                        start=True,
                        stop=False,
                    )
                    pss.append(ps)
                return pss

            def mm2s_and_out(gjs, pss, r0):
                for nj, ps in zip(gjs, pss):
                    c0 = nj * 512
                    nc.tensor.matmul(
                        ps[:],
                        LHS2[:, r0 : r0 + 128],
                        RHS2[:, c0 : c0 + 512],
                        start=False,
                        stop=True,
                    )
                g0 = gjs[0] * 512
                gw = len(gjs) * 512
                stage = stagep.tile([128, 4096], F16, tag="stage")
                for idx, (nj, ps) in enumerate(zip(gjs, pss)):
                    dst = stage[:, idx * 512 : (idx + 1) * 512]
                    if idx % 2 == 0:
                        nc.scalar.copy(dst, ps[:])
                    else:
                        nc.vector.tensor_copy(dst, ps[:])
                nc.sync.dma_start(
                    out=out[r0 : r0 + 128, g0 : g0 + gw], in_=stage[:, :gw]
                )

            # ---- preamble: sq1 + sq2 batch 0 claim their psum slots before
            #      mi0's mm1s fill the pool (any later allocation would wait
            #      on a main-loop copy that itself needs RHS2 -> deadlock).
            #      Batch 1 goes after mi0 so its bank reuse is acyclic. ----
            sq2_batch(0)
            pss0 = mm1s(grps[0], 0)
            mm2s_and_out(grps[0], pss0, 0)
            sq2_batch(1)

            # ---- main loop, grp outer / mi inner ----
            for gi, gjs in enumerate(grps):
                for mi in range(n_mb):
                    if gi == 0 and mi == 0:
                        continue  # emitted above
                    r0 = mi * 128
                    pss = mm1s(gjs, r0)
                    mm2s_and_out(gjs, pss, r0)
    return legalize_waits(nc) if legalize else nc


_NC_CACHE = {}


def _get_nc(ms=MS, n2=N2, d=D):
    key = (ms, n2, d)
    if key not in _NC_CACHE:
        _NC_CACHE[key] = build_nc(ms, n2, d)
    return _NC_CACHE[key]


def _prep_inputs(m1, m2, ms):
    """Host-side layout/precision prep (transpose + dtype casts only)."""
    bf16 = mybir.dt.np(BF16)
    m1ts = np.ascontiguousarray(-2.0 * m1.T).astype(bf16)  # [128, n1]
    m2t = np.ascontiguousarray(m2.T).astype(bf16)          # [128, n2]
    ncores = m1ts.shape[1] // ms
    return [
        {
            "m1ts": np.ascontiguousarray(m1ts[:, c * ms : (c + 1) * ms]),
            "m2t": m2t,
        }
        for c in range(ncores)
    ]


def kernel(mat_1, mat_2, _trace=False):
    m1 = np.ascontiguousarray(np.asarray(mat_1, dtype=np.float32))
    m2 = np.ascontiguousarray(np.asarray(mat_2, dtype=np.float32))
    assert m1.shape == (N1, D) and m2.shape == (N2, D)

    in_maps = _prep_inputs(m1, m2, MS)
    nc = _get_nc()
    r = run_bass_kernel_spmd(nc, in_maps, list(range(NCORES)), trace=_trace)
    out = np.concatenate(
        [r.results[c]["out"].astype(np.float32) for c in range(NCORES)], axis=0
    )
    if _trace:
        return out, r.exec_time_ns
    return out
